# revision 2
# baseline (speedup 1.0000x reference)
"""Trainium2 Bass kernel for nn_DecoderBlock (dense transformer decoder block).

Sharding: data-parallel over batch N=8 -> one batch element per NeuronCore.
Zero collectives; weights replicated to every core.

Per-core computation (K=1024 tokens, M=1024 emb, H=8 heads, DH=128, FF=4096):
  a1 = MHA(dec, dec);  x1 = LN(dec + a1)
  a2 = MHA(x1, enc);   x2 = LN(x1 + a2)
  ff = relu(x2 @ W1.T) @ W2.T;  out = LN(x2 + ff)

All matmuls run in float32r (fp32 rounded to 11 mantissa bits; full PE speed
at free-dim >= 256). LN/residual data stays fp32.

Layout strategy:
  - Activations kept in natural [k, m] layout for LN/residual (per-partition
    row stats) and in transposed [m, k] layout (f32r) as matmul operands.
  - Attention computes scores^T (keys on partitions). The softmax denominator
    comes from a ones-lhsT matmul that simultaneously broadcasts it to all
    128 partitions; y^T per head directly feeds the Wo matmul as lhsT, so the
    attention matrix is never transposed.
  - Only two PE-transpose passes exist (x1 -> x1T, x2 -> x2T).
Host pre-packs/transposes all weights so every DMA row is 2-4KB contiguous.
"""
import sys

sys.path.insert(0, "/opt/trn_rl_repo")

import numpy as np

# antenv.axon_hooks shim (needed only if BASS_TRACE is set; the agent image's
# read-only antenv package lacks this module).
try:
    from antenv import axon_hooks as _ah  # noqa: F401
except ImportError:
    import types as _types

    _h = _types.ModuleType("antenv.axon_hooks")
    _h._HOOK = None

    def _get_hook():
        if _h._HOOK is None:
            try:
                from trn_agent_boot.trn_boot import _ntff_profile_via_ctypes

                _h._HOOK = _ntff_profile_via_ctypes("/opt/axon/libaxon_pjrt.so")
            except Exception:
                _h._HOOK = None
        return _h._HOOK

    _h.get_axon_ntff_profile_hook = _get_hook
    _h.set_axon_ntff_profile_hook = lambda hook: setattr(_h, "_HOOK", hook)
    sys.modules["antenv.axon_hooks"] = _h

import concourse.bass as bass
import concourse.tile as tile
from concourse import bacc, mybir
from concourse.bass_utils import run_bass_kernel_spmd
from concourse.masks import make_identity

F32 = mybir.dt.float32
F32R = mybir.dt.float32r
AF = mybir.ActivationFunctionType
OP = mybir.AluOpType

P = 128          # partitions
K = 1024         # sequence length
M = 1024         # embedding dim
H = 8            # heads
DH = 128         # head dim
HD = H * DH      # 1024
FF = 4096
KT = K // P      # 8 seq tiles
MT = M // P      # 8 emb tiles
HT = HD // P     # 8 hd tiles
FT = FF // P     # 32 ff tiles
NQ = 4           # k_q quarters
QW = K // NQ     # 256
EPS = 1e-10
ISQ = 1.0 / float(np.sqrt(DH))

N_CORES = 8


def round_fp32r(x: np.ndarray) -> np.ndarray:
    """Round fp32 to fp32r (11-bit mantissa, RNE), fp32 container."""
    b = np.ascontiguousarray(x, dtype=np.float32).view(np.uint32)
    lsb = (b >> 12) & 1
    out = (b + 0x7FF + lsb) & 0xFFFFF000
    return out.view(np.float32)


def _bcast_row_ap(t: bass.AP, width: int) -> bass.AP:
    """DRAM vector -> AP broadcasting one row across 128 partitions."""
    return bass.AP(tensor=t.tensor, offset=t.offset, ap=[[0, P], [1, width]])


def build_kernel(flags: dict):
    nc = bacc.Bacc("TRN2", target_bir_lowering=False, debug=False,
                   num_devices=N_CORES)
    dram = {}

    def din(name, shape, dt=F32R):
        dram[name] = nc.dram_tensor(name, shape, dt, kind="ExternalInput").ap()

    din("xt_dec", (M, K))
    din("xt_enc", (M, K))
    din("wq_sa", (H, P, MT, DH)); din("wk_sa", (H, P, MT, DH))
    din("wq_ca", (H, P, MT, DH)); din("wk_ca", (H, P, MT, DH))
    din("wv_sa", (2, MT, P, HD // 2)); din("wv_ca", (2, MT, P, HD // 2))
    din("wo_sa", (HT, P, M)); din("wo_ca", (HT, P, M))
    din("w1", (FT, P, MT, P)); din("w2", (FT, P, M))
    din("ones", (P,))
    din("dec_nat", (K, M), F32)
    for nm in ("bq_sa", "bk_sa", "bq_ca", "bk_ca"):
        if flags[nm]:
            din(nm, (DH, H), F32)
    for nm in ("bv_sa", "bv_ca", "bo_sa", "bo_ca", "bf2",
               "g1", "b1", "g2", "b2", "g3", "b3"):
        if flags[nm]:
            din(nm, (M,), F32)
    if flags["bf1"]:
        din("bf1", (P, FT), F32)
    out = nc.dram_tensor("out", (K, M), F32, kind="ExternalOutput").ap()

    with tile.TileContext(nc) as tc:
        _emit(nc, tc, dram, out, flags)
    nc.compile()
    return nc


def _emit(nc, tc, dram, out, flags):
    from contextlib import ExitStack

    with ExitStack() as ctx:
        # ---------- persistent pools ----------
        const = ctx.enter_context(tc.tile_pool(name="const", bufs=1))
        wt = ctx.enter_context(tc.tile_pool(name="wt", bufs=3))
        natp = ctx.enter_context(tc.tile_pool(name="natp", bufs=2))
        residp = ctx.enter_context(tc.tile_pool(name="residp", bufs=2))
        statp = ctx.enter_context(tc.tile_pool(name="statp", bufs=4))
        xpool = ctx.enter_context(tc.tile_pool(name="xpool", bufs=1))
        ps = ctx.enter_context(tc.tile_pool(name="ps", bufs=8, space="PSUM"))
        dscr = ctx.enter_context(tc.tile_pool(name="dscr", bufs=1,
                                              space="DRAM"))

        ones_t = const.tile([P, P], F32R, name="ones_t")
        nc.sync.dma_start(out=ones_t, in_=_bcast_row_ap(dram["ones"], P))
        ident = const.tile([P, P], F32, name="ident")
        make_identity(nc, ident)
        eps_t = const.tile([P, 1], F32, name="eps_t")
        nc.vector.memset(eps_t, EPS)

        bias_tiles = {}
        for nm in ("bq_sa", "bk_sa", "bq_ca", "bk_ca"):
            if flags[nm]:
                t = const.tile([P, H], F32, name=nm + "_t")
                nc.sync.dma_start(out=t, in_=dram[nm])
                bias_tiles[nm] = t
        if flags["bf1"]:
            t = const.tile([P, FT], F32, name="bf1_t")
            nc.sync.dma_start(out=t, in_=dram["bf1"])
            bias_tiles["bf1"] = t
        for nm in ("bv_sa", "bv_ca", "bo_sa", "bo_ca", "bf2",
                   "g1", "b1", "g2", "b2", "g3", "b3"):
            if flags[nm]:
                t = const.tile([P, M], F32, name=nm + "_t")
                nc.sync.dma_start(out=t, in_=_bcast_row_ap(dram[nm], M))
                bias_tiles[nm] = t

        x1_store = dscr.tile([K, M], F32, name="x1_store")
        x2_store = dscr.tile([K, M], F32, name="x2_store")

        def new_xt(name):
            return xpool.tile([P, MT, K], F32R, name=name, tag="xt_slot")

        xt = new_xt("decT")
        nc.sync.dma_start(out=xt, in_=dram["xt_dec"].rearrange(
            "(mt p) k -> p mt k", p=P))

        # ---------------- building blocks ----------------
        def v_projection(src_xt, wv_name, bv_name, vcat):
            """vcat[p, kt, hd] (f32r) = src^T projected through Wv (+bv)."""
            for hh in range(2):
                with tc.tile_pool(name=f"wvp{hh}", bufs=8) as wvp:
                    wvts = []
                    for mt in range(MT):
                        w = wvp.tile([P, HD // 2], F32R,
                                     name=f"wv{hh}_{mt}", tag="wvh")
                        nc.sync.dma_start(out=w, in_=dram[wv_name][hh, mt])
                        wvts.append(w)
                    for kt in range(KT):
                        pv = ps.tile([P, 512], F32, name=f"psv{hh}_{kt}",
                                     tag="ps")
                        for mt in range(MT):
                            nc.tensor.matmul(
                                pv, src_xt[:, mt, kt * P:(kt + 1) * P],
                                wvts[mt], start=(mt == 0), stop=(mt == MT - 1))
                        dst = vcat[:, kt, hh * 512:(hh + 1) * 512]
                        if flags[bv_name]:
                            nc.vector.scalar_tensor_tensor(
                                out=dst, in0=pv, scalar=1.0,
                                in1=bias_tiles[bv_name][
                                    :, hh * 512:(hh + 1) * 512],
                                op0=OP.bypass, op1=OP.add)
                        else:
                            nc.scalar.copy(dst, pv)

        def qk_head_proj(src_xt, w_name, b_name, h, dst):
            """dst[p(d), k] (f32r) = head-h projection of src (+bias col)."""
            w = wt.tile([P, MT, DH], F32R, name=f"{w_name}_{h}", tag="wt")
            nc.sync.dma_start(out=w, in_=dram[w_name][h])
            for kh in range(2):
                pq = ps.tile([P, 512], F32, name=f"pq_{w_name}_{h}_{kh}",
                             tag="ps")
                for mt in range(MT):
                    nc.tensor.matmul(
                        pq, w[:, mt, :],
                        src_xt[:, mt, kh * 512:(kh + 1) * 512],
                        start=(mt == 0), stop=(mt == MT - 1))
                d = dst[:, kh * 512:(kh + 1) * 512]
                if b_name is not None and flags[b_name]:
                    nc.scalar.activation(d, pq, AF.Identity,
                                         bias=bias_tiles[b_name][:, h:h + 1])
                else:
                    nc.scalar.copy(d, pq)

        def attention(src_q_xt, kv_xt, wq_name, bq_name, wk_name, bk_name,
                      vcat, ycat):
            """ycat[p(d), ht, k] (f32r) = per-head softmax(qk^T/sqrt(d)) v."""
            with tc.tile_pool(name="attp", bufs=1) as attp:
                for h in range(H):
                    qh = attp.tile([P, K], F32R, name=f"qh_{h}", tag="qh",
                                   bufs=2)
                    qk_head_proj(src_q_xt, wq_name, bq_name, h, qh)
                    kh = attp.tile([P, K], F32R, name=f"kh_{h}", tag="kh",
                                   bufs=2)
                    qk_head_proj(kv_xt, wk_name, bk_name, h, kh)
                    for q in range(NQ):
                        expq = attp.tile([P, KT, QW], F32R,
                                         name=f"ex_{h}_{q}", tag="expq",
                                         bufs=2)
                        for kt in range(KT):
                            pss = ps.tile([P, QW], F32,
                                          name=f"pss{h}_{q}_{kt}", tag="ps")
                            nc.tensor.matmul(
                                pss, kh[:, kt * P:(kt + 1) * P],
                                qh[:, q * QW:(q + 1) * QW],
                                start=True, stop=True)
                            nc.scalar.activation(expq[:, kt, :], pss, AF.Exp,
                                                 scale=ISQ)
                        psd = ps.tile([P, QW], F32, name=f"psd{h}_{q}",
                                      tag="ps")
                        for kt in range(KT):
                            nc.tensor.matmul(psd, ones_t, expq[:, kt, :],
                                             start=(kt == 0),
                                             stop=(kt == KT - 1))
                        recip = attp.tile([P, QW], F32, name=f"rc_{h}_{q}",
                                          tag="recip", bufs=3)
                        nc.vector.reciprocal(recip, psd)
                        psy = ps.tile([P, QW], F32, name=f"psy{h}_{q}",
                                      tag="ps")
                        for kt in range(KT):
                            nc.tensor.matmul(
                                psy, vcat[:, kt, h * DH:(h + 1) * DH],
                                expq[:, kt, :], start=(kt == 0),
                                stop=(kt == KT - 1))
                        nc.vector.tensor_mul(
                            ycat[:, h, q * QW:(q + 1) * QW], psy, recip)

        def ln_tail(z, kt, g_name, b_name, store_dram, xt_new, to_out):
            """x = LN(z) (+g/b); DMA to scratch + transpose, or to output."""
            stats = statp.tile([P, 2, 6], F32, name=f"st{kt}", tag="stats")
            for sg in range(2):
                nc.vector.bn_stats(out=stats[:, sg, :],
                                   in_=z[:, sg * 512:(sg + 1) * 512])
            mv = statp.tile([P, 2], F32, name=f"mv{kt}", tag="mv")
            nc.vector.bn_aggr(out=mv, in_=stats)
            std = statp.tile([P, 1], F32, name=f"sd{kt}", tag="std")
            nc.scalar.activation(std, mv[:, 1:2], AF.Sqrt, bias=eps_t)
            inv = statp.tile([P, 1], F32, name=f"iv{kt}", tag="inv")
            nc.vector.reciprocal(inv, std)
            x = natp.tile([P, M], F32, name=f"x{kt}", tag="x")
            nc.vector.tensor_scalar(out=x, in0=z, scalar1=mv[:, 0:1],
                                    scalar2=inv, op0=OP.subtract, op1=OP.mult)
            if flags[g_name]:
                nc.vector.tensor_mul(x, x, bias_tiles[g_name])
            if flags[b_name]:
                nc.vector.tensor_add(x, x, bias_tiles[b_name])
            if to_out:
                nc.sync.dma_start(out=out[kt * P:(kt + 1) * P, :], in_=x)
                return
            nc.sync.dma_start(out=store_dram[kt * P:(kt + 1) * P, :], in_=x)
            for mt in range(MT):
                pt = ps.tile([P, P], F32, name=f"ptr{kt}_{mt}", tag="ps")
                nc.tensor.transpose(pt, x[:, mt * P:(mt + 1) * P], ident)
                nc.vector.tensor_copy(xt_new[:, mt, kt * P:(kt + 1) * P], pt)

        def wo_ln_block(ycat, wo_name, bo_name, resid_dram, g_name, b_name,
                        store_dram, xt_new):
            """a = ycat @ Wo^T (+bo); z = resid + a; LN tail per kt."""
            with tc.tile_pool(name="wop", bufs=8) as wop:
                wots = []
                for ht in range(HT):
                    w = wop.tile([P, M], F32R, name=f"{wo_name}{ht}",
                                 tag="wo8")
                    nc.sync.dma_start(out=w, in_=dram[wo_name][ht])
                    wots.append(w)
                for kt in range(KT):
                    resid = residp.tile([P, M], F32, name=f"rs{kt}",
                                        tag="resid")
                    nc.sync.dma_start(
                        out=resid, in_=resid_dram[kt * P:(kt + 1) * P, :])
                    z = natp.tile([P, M], F32, name=f"z{kt}", tag="z")
                    for mh in range(2):
                        pa = ps.tile([P, 512], F32, name=f"pa{kt}_{mh}",
                                     tag="ps")
                        for ht in range(HT):
                            nc.tensor.matmul(
                                pa, ycat[:, ht, kt * P:(kt + 1) * P],
                                wots[ht][:, mh * 512:(mh + 1) * 512],
                                start=(ht == 0), stop=(ht == HT - 1))
                        sl = slice(mh * 512, (mh + 1) * 512)
                        if flags[bo_name]:
                            nc.vector.scalar_tensor_tensor(
                                out=z[:, sl], in0=pa, scalar=1.0,
                                in1=bias_tiles[bo_name][:, sl],
                                op0=OP.bypass, op1=OP.add)
                            nc.vector.tensor_add(z[:, sl], z[:, sl],
                                                 resid[:, sl])
                        else:
                            nc.vector.tensor_add(z[:, sl], pa, resid[:, sl])
                    ln_tail(z, kt, g_name, b_name, store_dram, xt_new, False)

        # ================= self-attention =================
        with tc.tile_pool(name="sa_big", bufs=1) as bigp:
            vcat = bigp.tile([P, KT, HD], F32R, name="vcat_sa", tag="vcat")
            ycat = bigp.tile([P, HT, K], F32R, name="ycat_sa", tag="ycat")
            v_projection(xt, "wv_sa", "bv_sa", vcat)
            attention(xt, xt, "wq_sa", "bq_sa", "wk_sa", "bk_sa", vcat, ycat)
            x1t = new_xt("x1T")
            wo_ln_block(ycat, "wo_sa", "bo_sa", dram["dec_nat"], "g1", "b1",
                        x1_store, x1t)
        xt = x1t

        # ================= cross-attention =================
        with tc.tile_pool(name="ca_big", bufs=1) as bigp:
            vcat = bigp.tile([P, KT, HD], F32R, name="vcat_ca", tag="vcat")
            ycat = bigp.tile([P, HT, K], F32R, name="ycat_ca", tag="ycat")
            enc_xt = bigp.tile([P, MT, K], F32R, name="encT", tag="enct")
            nc.sync.dma_start(out=enc_xt, in_=dram["xt_enc"].rearrange(
                "(mt p) k -> p mt k", p=P))
            v_projection(enc_xt, "wv_ca", "bv_ca", vcat)
            attention(xt, enc_xt, "wq_ca", "bq_ca", "wk_ca", "bk_ca",
                      vcat, ycat)
            x2t = new_xt("x2T")
            wo_ln_block(ycat, "wo_ca", "bo_ca", x1_store, "g2", "b2",
                        x2_store, x2t)
        xt = x2t

        # ================= feed-forward =================
        with tc.tile_pool(name="ffp", bufs=1) as ffp:
            for kqh in range(2):
                rt = ffp.tile([P, FT, 512], F32R, name=f"rt{kqh}", tag="rt")
                for ft in range(FT):
                    w1t = wt.tile([P, MT, P], F32R, name=f"w1_{kqh}_{ft}",
                                  tag="wt")
                    nc.sync.dma_start(out=w1t, in_=dram["w1"][ft])
                    pf = ps.tile([P, 512], F32, name=f"pf{kqh}_{ft}", tag="ps")
                    for mt in range(MT):
                        nc.tensor.matmul(
                            pf, w1t[:, mt, :],
                            xt[:, mt, kqh * 512:(kqh + 1) * 512],
                            start=(mt == 0), stop=(mt == MT - 1))
                    if flags["bf1"]:
                        nc.scalar.activation(
                            rt[:, ft, :], pf, AF.Relu,
                            bias=bias_tiles["bf1"][:, ft:ft + 1])
                    else:
                        nc.scalar.activation(rt[:, ft, :], pf, AF.Relu)
                x2r, z3 = [], []
                for ks in range(4):
                    kt = kqh * 4 + ks
                    r = ffp.tile([P, M], F32, name=f"x2r{kt}", tag="x2r",
                                 bufs=4)
                    nc.sync.dma_start(
                        out=r, in_=x2_store[kt * P:(kt + 1) * P, :])
                    x2r.append(r)
                    z3.append(ffp.tile([P, M], F32, name=f"z3_{kt}",
                                       tag="z3", bufs=4))
                for mq in range(4):
                    pacc = [ps.tile([P, 256], F32, name=f"po{kqh}_{mq}_{ks}",
                                    tag="ps") for ks in range(4)]
                    for ft in range(FT):
                        w2t = wt.tile([P, 256], F32R,
                                      name=f"w2_{kqh}_{mq}_{ft}", tag="wt2")
                        nc.sync.dma_start(
                            out=w2t,
                            in_=dram["w2"][ft, :, mq * 256:(mq + 1) * 256])
                        for ks in range(4):
                            nc.tensor.matmul(
                                pacc[ks], rt[:, ft, ks * P:(ks + 1) * P], w2t,
                                start=(ft == 0), stop=(ft == FT - 1))
                    sl = slice(mq * 256, (mq + 1) * 256)
                    for ks in range(4):
                        if flags["bf2"]:
                            nc.vector.scalar_tensor_tensor(
                                out=z3[ks][:, sl], in0=pacc[ks], scalar=1.0,
                                in1=bias_tiles["bf2"][:, sl],
                                op0=OP.bypass, op1=OP.add)
                            nc.vector.tensor_add(z3[ks][:, sl], z3[ks][:, sl],
                                                 x2r[ks][:, sl])
                        else:
                            nc.vector.tensor_add(z3[ks][:, sl], pacc[ks],
                                                 x2r[ks][:, sl])
                for ks in range(4):
                    ln_tail(z3[ks], kqh * 4 + ks, "g3", "b3", None, None,
                            True)


def _pack_inputs(inputs: dict):
    """Host-side packing -> (flags, per-core in_maps)."""
    f32 = np.float32
    dec = np.asarray(inputs["dec"], f32)
    enc = np.asarray(inputs["enc"], f32)

    def nz(x):
        return bool(np.any(np.asarray(x) != 0.0))

    flags = {
        "bq_sa": nz(inputs["bq_sa"]), "bk_sa": nz(inputs["bk_sa"]),
        "bv_sa": nz(inputs["bv_sa"]), "bo_sa": nz(inputs["bo_sa"]),
        "bq_ca": nz(inputs["bq_ca"]), "bk_ca": nz(inputs["bk_ca"]),
        "bv_ca": nz(inputs["bv_ca"]), "bo_ca": nz(inputs["bo_ca"]),
        "bf1": nz(inputs["bf1"]), "bf2": nz(inputs["bf2"]),
        "g1": bool(np.any(np.asarray(inputs["g1"]) != 1.0)),
        "b1": nz(inputs["b1"]),
        "g2": bool(np.any(np.asarray(inputs["g2"]) != 1.0)),
        "b2": nz(inputs["b2"]),
        "g3": bool(np.any(np.asarray(inputs["g3"]) != 1.0)),
        "b3": nz(inputs["b3"]),
    }

    def qk_pack(w):
        w = np.asarray(w, f32)  # (H, DH, M)
        return round_fp32r(
            w.transpose(0, 2, 1).reshape(H, MT, P, DH).transpose(0, 2, 1, 3))

    def v_pack(w):
        w = np.asarray(w, f32)  # (H, DH, M) -> WvT [m, hd]
        wt_ = w.transpose(2, 0, 1).reshape(M, HD)
        return round_fp32r(
            wt_.reshape(MT, P, 2, HD // 2).transpose(2, 0, 1, 3))

    def o_pack(w):  # (M, HD) -> WoT (HD, M) -> (HT, P, M)
        return round_fp32r(
            np.ascontiguousarray(np.asarray(w, f32).T).reshape(HT, P, M))

    W1 = np.asarray(inputs["W1"], f32)
    W2 = np.asarray(inputs["W2"], f32)
    shared = {
        "wq_sa": qk_pack(inputs["Wq_sa"]), "wk_sa": qk_pack(inputs["Wk_sa"]),
        "wv_sa": v_pack(inputs["Wv_sa"]), "wo_sa": o_pack(inputs["Wo_sa"]),
        "wq_ca": qk_pack(inputs["Wq_ca"]), "wk_ca": qk_pack(inputs["Wk_ca"]),
        "wv_ca": v_pack(inputs["Wv_ca"]), "wo_ca": o_pack(inputs["Wo_ca"]),
        "w1": round_fp32r(W1.reshape(FT, P, MT, P).transpose(0, 3, 2, 1)),
        "w2": round_fp32r(np.ascontiguousarray(W2.T).reshape(FT, P, M)),
        "ones": np.ones(P, f32),
    }
    for nm in ("bq_sa", "bk_sa", "bq_ca", "bk_ca"):
        if flags[nm]:
            shared[nm] = np.ascontiguousarray(np.asarray(inputs[nm], f32).T)
    for nm in ("bv_sa", "bv_ca"):
        if flags[nm]:
            shared[nm] = np.asarray(inputs[nm], f32).reshape(HD)
    for nm in ("bo_sa", "bo_ca", "bf2", "g1", "b1", "g2", "b2", "g3", "b3"):
        if flags[nm]:
            shared[nm] = np.asarray(inputs[nm], f32)
    if flags["bf1"]:
        shared["bf1"] = np.ascontiguousarray(
            np.asarray(inputs["bf1"], f32).reshape(FT, P).T)

    in_maps = []
    for c in range(N_CORES):
        m = dict(shared)
        m["xt_dec"] = round_fp32r(np.ascontiguousarray(dec[c].T))
        m["xt_enc"] = round_fp32r(np.ascontiguousarray(enc[c].T))
        m["dec_nat"] = np.ascontiguousarray(dec[c])
        in_maps.append(m)
    return flags, in_maps


_NC_CACHE: dict = {}


def kernel(**inputs) -> np.ndarray:
    flags, in_maps = _pack_inputs(inputs)
    key = tuple(sorted(flags.items()))
    if key not in _NC_CACHE:
        _NC_CACHE[key] = build_kernel(flags)
    nc = _NC_CACHE[key]
    res = run_bass_kernel_spmd(nc, in_maps, core_ids=list(range(N_CORES)))
    return np.stack([res.results[c]["out"] for c in range(N_CORES)])


# revision 9
# speedup vs baseline: 1.1339x; 1.1339x over previous
"""Trainium2 Bass kernel for nn_DecoderBlock (dense transformer decoder block).

Sharding: data-parallel over batch N=8 -> one batch element per NeuronCore.
Zero collectives; weights replicated to every core.

Per-core computation (K=1024 tokens, M=1024 emb, H=8 heads, DH=128, FF=4096):
  a1 = MHA(dec, dec);  x1 = LN(dec + a1)
  a2 = MHA(x1, enc);   x2 = LN(x1 + a2)
  ff = relu(x2 @ W1.T) @ W2.T;  out = LN(x2 + ff)

All matmuls run in float32r (fp32 rounded to 11 mantissa bits; full PE speed
at free-dim >= 256). LN/residual data stays fp32.

Layout strategy:
  - Activations kept in natural [k, m] layout for LN/residual (per-partition
    row stats) and in transposed [m, k] layout (f32r) as matmul operands.
  - Attention computes scores^T (keys on partitions). The softmax denominator
    comes from a ones-lhsT matmul that simultaneously broadcasts it to all
    128 partitions; y^T per head directly feeds the Wo matmul as lhsT, so the
    attention matrix is never transposed.
  - Only two PE-transpose passes exist (x1 -> x1T, x2 -> x2T).
Host pre-packs/transposes all weights so every DMA row is 2-4KB contiguous.
"""
import sys

sys.path.insert(0, "/opt/trn_rl_repo")

import numpy as np

# antenv.axon_hooks shim (needed only if BASS_TRACE is set; the agent image's
# read-only antenv package lacks this module).
try:
    from antenv import axon_hooks as _ah  # noqa: F401
except ImportError:
    import types as _types

    _h = _types.ModuleType("antenv.axon_hooks")
    _h._HOOK = None

    def _get_hook():
        if _h._HOOK is None:
            try:
                from trn_agent_boot.trn_boot import _ntff_profile_via_ctypes

                _h._HOOK = _ntff_profile_via_ctypes("/opt/axon/libaxon_pjrt.so")
            except Exception:
                _h._HOOK = None
        return _h._HOOK

    _h.get_axon_ntff_profile_hook = _get_hook
    _h.set_axon_ntff_profile_hook = lambda hook: setattr(_h, "_HOOK", hook)
    sys.modules["antenv.axon_hooks"] = _h

import concourse.bass as bass
import concourse.tile as tile
from concourse import bacc, mybir
from concourse.bass_utils import run_bass_kernel_spmd
from concourse.masks import make_identity

F32 = mybir.dt.float32
F32R = mybir.dt.float32r
AF = mybir.ActivationFunctionType
OP = mybir.AluOpType

P = 128          # partitions
K = 1024         # sequence length
M = 1024         # embedding dim
H = 8            # heads
DH = 128         # head dim
HD = H * DH      # 1024
FF = 4096
KT = K // P      # 8 seq tiles
MT = M // P      # 8 emb tiles
HT = HD // P     # 8 hd tiles
FT = FF // P     # 32 ff tiles
NQ = 2           # k_q halves (free dim 512 keeps fp32r at full PE speed)
QW = K // NQ     # 512
EPS = 1e-10
ISQ = 1.0 / float(np.sqrt(DH))

N_CORES = 8


def round_fp32r(x: np.ndarray) -> np.ndarray:
    """Round fp32 to fp32r (11-bit mantissa, RNE), fp32 container."""
    b = np.ascontiguousarray(x, dtype=np.float32).view(np.uint32)
    lsb = (b >> 12) & 1
    out = (b + 0x7FF + lsb) & 0xFFFFF000
    return out.view(np.float32)


def _bcast_row_ap(t: bass.AP, width: int) -> bass.AP:
    """DRAM vector -> AP broadcasting one row across 128 partitions."""
    return bass.AP(tensor=t.tensor, offset=t.offset, ap=[[0, P], [1, width]])


def build_kernel(flags: dict):
    nc = bacc.Bacc("TRN2", target_bir_lowering=False, debug=False,
                   num_devices=N_CORES)
    dram = {}

    def din(name, shape, dt=F32R):
        dram[name] = nc.dram_tensor(name, shape, dt, kind="ExternalInput").ap()

    din("xt_dec", (M, K))
    din("xt_enc", (M, K))
    din("wq_sa", (H, P, MT, DH)); din("wk_sa", (H, P, MT, DH))
    din("wq_ca", (H, P, MT, DH)); din("wk_ca", (H, P, MT, DH))
    din("wv_sa", (2, MT, P, HD // 2)); din("wv_ca", (2, MT, P, HD // 2))
    din("wo_sa", (HT, P, M)); din("wo_ca", (HT, P, M))
    din("w1", (FT, P, MT, P)); din("w2", (FT, P, M))
    din("ones", (P,))
    din("dec_nat", (K, M), F32)
    for nm in ("bq_sa", "bk_sa", "bq_ca", "bk_ca"):
        if flags[nm]:
            din(nm, (DH, H), F32)
    for nm in ("bv_sa", "bv_ca", "bo_sa", "bo_ca", "bf2",
               "g1", "b1", "g2", "b2", "g3", "b3"):
        if flags[nm]:
            din(nm, (M,), F32)
    if flags["bf1"]:
        din("bf1", (P, FT), F32)
    out = nc.dram_tensor("out", (K, M), F32, kind="ExternalOutput").ap()

    with tile.TileContext(nc) as tc:
        _emit(nc, tc, dram, out, flags)
    nc.compile()
    return nc


def _emit(nc, tc, dram, out, flags):
    from contextlib import ExitStack

    with ExitStack() as ctx:
        # ---------- persistent pools ----------
        const = ctx.enter_context(tc.tile_pool(name="const", bufs=1))
        wt = ctx.enter_context(tc.tile_pool(name="wt", bufs=3))
        natp = ctx.enter_context(tc.tile_pool(name="natp", bufs=2))
        residp = ctx.enter_context(tc.tile_pool(name="residp", bufs=2))
        statp = ctx.enter_context(tc.tile_pool(name="statp", bufs=4))
        xpool = ctx.enter_context(tc.tile_pool(name="xpool", bufs=1))
        ps = ctx.enter_context(tc.tile_pool(name="ps", bufs=8, space="PSUM"))
        dscr = ctx.enter_context(tc.tile_pool(name="dscr", bufs=1,
                                              space="DRAM"))

        ones_t = const.tile([P, P], F32R, name="ones_t")
        nc.sync.dma_start(out=ones_t, in_=_bcast_row_ap(dram["ones"], P))
        ident = const.tile([P, P], F32, name="ident")
        make_identity(nc, ident)
        eps_t = const.tile([P, 1], F32, name="eps_t")
        nc.vector.memset(eps_t, EPS)

        bias_tiles = {}
        for nm in ("bq_sa", "bk_sa", "bq_ca", "bk_ca"):
            if flags[nm]:
                t = const.tile([P, H], F32, name=nm + "_t")
                nc.sync.dma_start(out=t, in_=dram[nm])
                bias_tiles[nm] = t
        if flags["bf1"]:
            t = const.tile([P, FT], F32, name="bf1_t")
            nc.sync.dma_start(out=t, in_=dram["bf1"])
            bias_tiles["bf1"] = t
        for nm in ("bv_sa", "bv_ca", "bo_sa", "bo_ca", "bf2",
                   "g1", "b1", "g2", "b2", "g3", "b3"):
            if flags[nm]:
                t = const.tile([P, M], F32, name=nm + "_t")
                nc.sync.dma_start(out=t, in_=_bcast_row_ap(dram[nm], M))
                bias_tiles[nm] = t

        x1_store = dscr.tile([K, M], F32, name="x1_store")
        x2_store = dscr.tile([K, M], F32, name="x2_store")

        def new_xt(name):
            return xpool.tile([P, MT, K], F32R, name=name, tag="xt_slot")

        xt = new_xt("decT")
        nc.sync.dma_start(out=xt, in_=dram["xt_dec"].rearrange(
            "(mt p) k -> p mt k", p=P))
        encp = ctx.enter_context(tc.tile_pool(name="encp", bufs=1))
        enc_xt = encp.tile([P, MT, K], F32R, name="encT", tag="enct")
        nc.sync.dma_start(out=enc_xt, in_=dram["xt_enc"].rearrange(
            "(mt p) k -> p mt k", p=P))

        # ---------------- building blocks ----------------
        def v_projection(src_xt, wv_name, bv_name, vcat):
            """vcat[p, kt, hd] (f32r) = src^T projected through Wv (+bv)."""
            for hh in range(2):
                with tc.tile_pool(name=f"wvp{hh}", bufs=8) as wvp:
                    wvts = []
                    for mt in range(MT):
                        w = wvp.tile([P, HD // 2], F32R,
                                     name=f"wv{hh}_{mt}", tag="wvh")
                        nc.sync.dma_start(out=w, in_=dram[wv_name][hh, mt])
                        wvts.append(w)
                    for kt in range(KT):
                        pv = ps.tile([P, 512], F32, name=f"psv{hh}_{kt}",
                                     tag="ps")
                        for mt in range(MT):
                            nc.tensor.matmul(
                                pv, src_xt[:, mt, kt * P:(kt + 1) * P],
                                wvts[mt], start=(mt == 0), stop=(mt == MT - 1))
                        dst = vcat[:, kt, hh * 512:(hh + 1) * 512]
                        if flags[bv_name]:
                            nc.vector.scalar_tensor_tensor(
                                out=dst, in0=pv, scalar=1.0,
                                in1=bias_tiles[bv_name][
                                    :, hh * 512:(hh + 1) * 512],
                                op0=OP.bypass, op1=OP.add)
                        else:
                            nc.scalar.copy(dst, pv)

        def qk_head_proj(src_xt, w_name, b_name, h, dst):
            """dst[p(d), k] (f32r) = head-h projection of src (+bias col)."""
            w = wt.tile([P, MT, DH], F32R, name=f"{w_name}_{h}", tag="wt")
            nc.sync.dma_start(out=w, in_=dram[w_name][h])
            for kh in range(2):
                pq = ps.tile([P, 512], F32, name=f"pq_{w_name}_{h}_{kh}",
                             tag="ps")
                for mt in range(MT):
                    nc.tensor.matmul(
                        pq, w[:, mt, :],
                        src_xt[:, mt, kh * 512:(kh + 1) * 512],
                        start=(mt == 0), stop=(mt == MT - 1))
                d = dst[:, kh * 512:(kh + 1) * 512]
                if b_name is not None and flags[b_name]:
                    nc.scalar.activation(d, pq, AF.Identity,
                                         bias=bias_tiles[b_name][:, h:h + 1])
                else:
                    nc.scalar.copy(d, pq)

        def attention(src_q_xt, kv_xt, wq_name, bq_name, wk_name, bk_name,
                      vcat, ycat):
            """ycat[p(d), ht, k] (f32r) = per-head softmax(qk^T/sqrt(d)) v."""
            with tc.tile_pool(name="attp", bufs=1) as attp:
                for h in range(H):
                    qh = attp.tile([P, K], F32R, name=f"qh_{h}", tag="qh",
                                   bufs=2)
                    qk_head_proj(src_q_xt, wq_name, bq_name, h, qh)
                    kh = attp.tile([P, K], F32R, name=f"kh_{h}", tag="kh",
                                   bufs=1)
                    qk_head_proj(kv_xt, wk_name, bk_name, h, kh)
                    for q in range(NQ):
                        expq = attp.tile([P, KT, QW], F32R,
                                         name=f"ex_{h}_{q}", tag="expq",
                                         bufs=1)
                        for kt in range(KT):
                            pss = ps.tile([P, QW], F32,
                                          name=f"pss{h}_{q}_{kt}", tag="ps")
                            nc.tensor.matmul(
                                pss, kh[:, kt * P:(kt + 1) * P],
                                qh[:, q * QW:(q + 1) * QW],
                                start=True, stop=True)
                            nc.scalar.activation(expq[:, kt, :], pss, AF.Exp,
                                                 scale=ISQ)
                        psd = ps.tile([P, QW], F32, name=f"psd{h}_{q}",
                                      tag="ps")
                        for kt in range(KT):
                            nc.tensor.matmul(psd, ones_t, expq[:, kt, :],
                                             start=(kt == 0),
                                             stop=(kt == KT - 1))
                        recip = attp.tile([P, QW], F32, name=f"rc_{h}_{q}",
                                          tag="recip", bufs=2)
                        nc.vector.reciprocal(recip, psd)
                        psy = ps.tile([P, QW], F32, name=f"psy{h}_{q}",
                                      tag="ps")
                        for kt in range(KT):
                            nc.tensor.matmul(
                                psy, vcat[:, kt, h * DH:(h + 1) * DH],
                                expq[:, kt, :], start=(kt == 0),
                                stop=(kt == KT - 1))
                        nc.vector.tensor_mul(
                            ycat[:, h, q * QW:(q + 1) * QW], psy, recip)

        def ln_tail(z, kt, g_name, b_name, store_dram, xt_new, to_out):
            """x = LN(z) (+g/b); DMA to scratch + transpose, or to output."""
            stats = statp.tile([P, 2, 6], F32, name=f"st{kt}", tag="stats")
            for sg in range(2):
                nc.vector.bn_stats(out=stats[:, sg, :],
                                   in_=z[:, sg * 512:(sg + 1) * 512])
            mv = statp.tile([P, 2], F32, name=f"mv{kt}", tag="mv")
            nc.vector.bn_aggr(out=mv, in_=stats)
            std = statp.tile([P, 1], F32, name=f"sd{kt}", tag="std")
            nc.scalar.activation(std, mv[:, 1:2], AF.Sqrt, bias=eps_t)
            inv = statp.tile([P, 1], F32, name=f"iv{kt}", tag="inv")
            nc.vector.reciprocal(inv, std)
            x = natp.tile([P, M], F32, name=f"x{kt}", tag="x")
            nc.vector.tensor_scalar(out=x, in0=z, scalar1=mv[:, 0:1],
                                    scalar2=inv, op0=OP.subtract, op1=OP.mult)
            if flags[g_name]:
                nc.vector.tensor_mul(x, x, bias_tiles[g_name])
            if flags[b_name]:
                nc.vector.tensor_add(x, x, bias_tiles[b_name])
            if to_out:
                nc.sync.dma_start(out=out[kt * P:(kt + 1) * P, :], in_=x)
                return
            nc.sync.dma_start(out=store_dram[kt * P:(kt + 1) * P, :], in_=x)
            for mt in range(MT):
                pt = ps.tile([P, P], F32, name=f"ptr{kt}_{mt}", tag="ps")
                nc.tensor.transpose(pt, x[:, mt * P:(mt + 1) * P], ident)
                nc.vector.tensor_copy(xt_new[:, mt, kt * P:(kt + 1) * P], pt)

        def wo_ln_block(ycat, wo_name, bo_name, resid_dram, g_name, b_name,
                        store_dram, xt_new):
            """a = ycat @ Wo^T (+bo); z = resid + a; LN tail per kt."""
            with tc.tile_pool(name="wop", bufs=8) as wop:
                wots = []
                for ht in range(HT):
                    w = wop.tile([P, M], F32R, name=f"{wo_name}{ht}",
                                 tag="wo8")
                    nc.sync.dma_start(out=w, in_=dram[wo_name][ht])
                    wots.append(w)
                for kt in range(KT):
                    resid = residp.tile([P, M], F32, name=f"rs{kt}",
                                        tag="resid")
                    nc.sync.dma_start(
                        out=resid, in_=resid_dram[kt * P:(kt + 1) * P, :])
                    z = natp.tile([P, M], F32, name=f"z{kt}", tag="z")
                    for mh in range(2):
                        pa = ps.tile([P, 512], F32, name=f"pa{kt}_{mh}",
                                     tag="ps")
                        for ht in range(HT):
                            nc.tensor.matmul(
                                pa, ycat[:, ht, kt * P:(kt + 1) * P],
                                wots[ht][:, mh * 512:(mh + 1) * 512],
                                start=(ht == 0), stop=(ht == HT - 1))
                        sl = slice(mh * 512, (mh + 1) * 512)
                        if flags[bo_name]:
                            nc.vector.scalar_tensor_tensor(
                                out=z[:, sl], in0=pa, scalar=1.0,
                                in1=bias_tiles[bo_name][:, sl],
                                op0=OP.bypass, op1=OP.add)
                            nc.vector.tensor_add(z[:, sl], z[:, sl],
                                                 resid[:, sl])
                        else:
                            nc.vector.tensor_add(z[:, sl], pa, resid[:, sl])
                    ln_tail(z, kt, g_name, b_name, store_dram, xt_new, False)

        # ================= self-attention =================
        with tc.tile_pool(name="sa_big", bufs=1) as bigp:
            vcat = bigp.tile([P, KT, HD], F32R, name="vcat_sa", tag="vcat")
            ycat = bigp.tile([P, HT, K], F32R, name="ycat_sa", tag="ycat")
            v_projection(xt, "wv_sa", "bv_sa", vcat)
            attention(xt, xt, "wq_sa", "bq_sa", "wk_sa", "bk_sa", vcat, ycat)
            x1t = new_xt("x1T")
            wo_ln_block(ycat, "wo_sa", "bo_sa", dram["dec_nat"], "g1", "b1",
                        x1_store, x1t)
        xt = x1t

        # ================= cross-attention =================
        with tc.tile_pool(name="ca_big", bufs=1) as bigp:
            vcat = bigp.tile([P, KT, HD], F32R, name="vcat_ca", tag="vcat")
            ycat = bigp.tile([P, HT, K], F32R, name="ycat_ca", tag="ycat")
            v_projection(enc_xt, "wv_ca", "bv_ca", vcat)
            attention(xt, enc_xt, "wq_ca", "bq_ca", "wk_ca", "bk_ca",
                      vcat, ycat)
            x2t = new_xt("x2T")
            wo_ln_block(ycat, "wo_ca", "bo_ca", x1_store, "g2", "b2",
                        x2_store, x2t)
        xt = x2t

        # ================= feed-forward =================
        with tc.tile_pool(name="ffp", bufs=1) as ffp:
            for kqh in range(2):
                rt = ffp.tile([P, FT, 512], F32R, name=f"rt{kqh}", tag="rt")
                for ft in range(FT):
                    w1t = wt.tile([P, MT, P], F32R, name=f"w1_{kqh}_{ft}",
                                  tag="wt")
                    nc.sync.dma_start(out=w1t, in_=dram["w1"][ft])
                    pf = ps.tile([P, 512], F32, name=f"pf{kqh}_{ft}", tag="ps")
                    for mt in range(MT):
                        nc.tensor.matmul(
                            pf, w1t[:, mt, :],
                            xt[:, mt, kqh * 512:(kqh + 1) * 512],
                            start=(mt == 0), stop=(mt == MT - 1))
                    if flags["bf1"]:
                        nc.scalar.activation(
                            rt[:, ft, :], pf, AF.Relu,
                            bias=bias_tiles["bf1"][:, ft:ft + 1])
                    else:
                        nc.scalar.activation(rt[:, ft, :], pf, AF.Relu)
                x2r, z3 = [], []
                for ks in range(4):
                    kt = kqh * 4 + ks
                    r = ffp.tile([P, M], F32, name=f"x2r{kt}", tag="x2r",
                                 bufs=4)
                    nc.sync.dma_start(
                        out=r, in_=x2_store[kt * P:(kt + 1) * P, :])
                    x2r.append(r)
                    z3.append(ffp.tile([P, M], F32, name=f"z3_{kt}",
                                       tag="z3", bufs=4))
                for mh in range(2):
                    pacc = [ps.tile([P, 512], F32, name=f"po{kqh}_{mh}_{ks}",
                                    tag="ps") for ks in range(4)]
                    for ft in range(FT):
                        w2t = wt.tile([P, 512], F32R,
                                      name=f"w2_{kqh}_{mh}_{ft}", tag="wt2")
                        nc.sync.dma_start(
                            out=w2t,
                            in_=dram["w2"][ft, :, mh * 512:(mh + 1) * 512])
                        for ks in range(4):
                            nc.tensor.matmul(
                                pacc[ks], rt[:, ft, ks * P:(ks + 1) * P], w2t,
                                start=(ft == 0), stop=(ft == FT - 1))
                    sl = slice(mh * 512, (mh + 1) * 512)
                    for ks in range(4):
                        if flags["bf2"]:
                            nc.vector.scalar_tensor_tensor(
                                out=z3[ks][:, sl], in0=pacc[ks], scalar=1.0,
                                in1=bias_tiles["bf2"][:, sl],
                                op0=OP.bypass, op1=OP.add)
                            nc.vector.tensor_add(z3[ks][:, sl], z3[ks][:, sl],
                                                 x2r[ks][:, sl])
                        else:
                            nc.vector.tensor_add(z3[ks][:, sl], pacc[ks],
                                                 x2r[ks][:, sl])
                for ks in range(4):
                    ln_tail(z3[ks], kqh * 4 + ks, "g3", "b3", None, None,
                            True)


def _pack_inputs(inputs: dict):
    """Host-side packing -> (flags, per-core in_maps)."""
    f32 = np.float32
    dec = np.asarray(inputs["dec"], f32)
    enc = np.asarray(inputs["enc"], f32)

    def nz(x):
        return bool(np.any(np.asarray(x) != 0.0))

    flags = {
        "bq_sa": nz(inputs["bq_sa"]), "bk_sa": nz(inputs["bk_sa"]),
        "bv_sa": nz(inputs["bv_sa"]), "bo_sa": nz(inputs["bo_sa"]),
        "bq_ca": nz(inputs["bq_ca"]), "bk_ca": nz(inputs["bk_ca"]),
        "bv_ca": nz(inputs["bv_ca"]), "bo_ca": nz(inputs["bo_ca"]),
        "bf1": nz(inputs["bf1"]), "bf2": nz(inputs["bf2"]),
        "g1": bool(np.any(np.asarray(inputs["g1"]) != 1.0)),
        "b1": nz(inputs["b1"]),
        "g2": bool(np.any(np.asarray(inputs["g2"]) != 1.0)),
        "b2": nz(inputs["b2"]),
        "g3": bool(np.any(np.asarray(inputs["g3"]) != 1.0)),
        "b3": nz(inputs["b3"]),
    }

    def qk_pack(w):
        w = np.asarray(w, f32)  # (H, DH, M)
        return round_fp32r(
            w.transpose(0, 2, 1).reshape(H, MT, P, DH).transpose(0, 2, 1, 3))

    def v_pack(w):
        w = np.asarray(w, f32)  # (H, DH, M) -> WvT [m, hd]
        wt_ = w.transpose(2, 0, 1).reshape(M, HD)
        return round_fp32r(
            wt_.reshape(MT, P, 2, HD // 2).transpose(2, 0, 1, 3))

    def o_pack(w):  # (M, HD) -> WoT (HD, M) -> (HT, P, M)
        return round_fp32r(
            np.ascontiguousarray(np.asarray(w, f32).T).reshape(HT, P, M))

    W1 = np.asarray(inputs["W1"], f32)
    W2 = np.asarray(inputs["W2"], f32)
    shared = {
        "wq_sa": qk_pack(inputs["Wq_sa"]), "wk_sa": qk_pack(inputs["Wk_sa"]),
        "wv_sa": v_pack(inputs["Wv_sa"]), "wo_sa": o_pack(inputs["Wo_sa"]),
        "wq_ca": qk_pack(inputs["Wq_ca"]), "wk_ca": qk_pack(inputs["Wk_ca"]),
        "wv_ca": v_pack(inputs["Wv_ca"]), "wo_ca": o_pack(inputs["Wo_ca"]),
        "w1": round_fp32r(W1.reshape(FT, P, MT, P).transpose(0, 3, 2, 1)),
        "w2": round_fp32r(np.ascontiguousarray(W2.T).reshape(FT, P, M)),
        "ones": np.ones(P, f32),
    }
    for nm in ("bq_sa", "bk_sa", "bq_ca", "bk_ca"):
        if flags[nm]:
            shared[nm] = np.ascontiguousarray(np.asarray(inputs[nm], f32).T)
    for nm in ("bv_sa", "bv_ca"):
        if flags[nm]:
            shared[nm] = np.asarray(inputs[nm], f32).reshape(HD)
    for nm in ("bo_sa", "bo_ca", "bf2", "g1", "b1", "g2", "b2", "g3", "b3"):
        if flags[nm]:
            shared[nm] = np.asarray(inputs[nm], f32)
    if flags["bf1"]:
        shared["bf1"] = np.ascontiguousarray(
            np.asarray(inputs["bf1"], f32).reshape(FT, P).T)

    in_maps = []
    for c in range(N_CORES):
        m = dict(shared)
        m["xt_dec"] = round_fp32r(np.ascontiguousarray(dec[c].T))
        m["xt_enc"] = round_fp32r(np.ascontiguousarray(enc[c].T))
        m["dec_nat"] = np.ascontiguousarray(dec[c])
        in_maps.append(m)
    return flags, in_maps


_NC_CACHE: dict = {}


def kernel(**inputs) -> np.ndarray:
    flags, in_maps = _pack_inputs(inputs)
    key = tuple(sorted(flags.items()))
    if key not in _NC_CACHE:
        _NC_CACHE[key] = build_kernel(flags)
    nc = _NC_CACHE[key]
    res = run_bass_kernel_spmd(nc, in_maps, core_ids=list(range(N_CORES)))
    return np.stack([res.results[c]["out"] for c in range(N_CORES)])
